# revision 33
# baseline (speedup 1.0000x reference)
"""Trainium2 Bass kernel for nn_AttentionModel (image-caption LSTM with
pre-computable 'attention').

Math refactor (exact, weight-level):
    feats = images @ W_fc.T + b_fc
    ctx_t = [feats, emb_t] @ W_att.T + b_att
    x_t   = [feats, ctx_t]
    gates = x_t @ W_ih.T + b_ih + h @ W_hh.T + b_hh
Since ctx_t does not depend on h, fold the attention layer into effective
weights (host-side, fp32):
    gates_x_t = feats @ W1 + emb_t @ W2 + b_eff
      W1   = W_ih[:, :256].T + W_att[:, :256].T @ W_ih[:, 256:].T   [256, 2048]
      W2   = W_att[:, 256:].T @ W_ih[:, 256:].T                     [512, 2048]
      b_eff= W_ih[:, 256:] @ b_att + b_ih + b_hh                    [2048]
Only the h @ W_hh.T term is recurrent.

The caption->gates product is further rank-factored (W2 = W_att_e.T @
W_ih_c.T has rank 256): stage1 computes ctx_eT = W_att_e @ emb.T once per
timestep pair, stage2 multiplies by W_ih_c.T.

Device layout (per core, batch shard = 32):
  - gates PSUM, two banks per step (hidden halves of 256): each
    [128, 256] with partition groups of 32 = (i, f, o, g), free dim =
    hidden half.  Produced by col-tiled matmuls (tile_position=(0,32j)):
    gx0-seed via an identity matmul, 2 stage2 K-rounds (h-independent),
    4 W_hh K-rounds (recurrent).  Halving lets sigma/cell of half A
    overlap the PE rounds of half B and the next step's h-rounds.
  - LSTM cell on ScalarE sigmoid (one op for all gates; tanh(g) =
    2*sigmoid(2g)-1 with g pre-scaled by 2 on the host) + VectorE, bf16.
  - h transposed back to [hidden, batch] via PE transposes per half and
    stored into hs_all; the W_out projection is interleaved into the
    recurrence one (chunk, vocab-tile) unit per step so its matmuls fill
    PE idle slots.
"""

import sys

sys.path.insert(0, "/opt/trn_rl_repo")

import numpy as np
import ml_dtypes

import concourse.bass as bass
import concourse.tile as tile
from concourse import bacc, mybir

BF16 = mybir.dt.bfloat16
F32 = mybir.dt.float32

B, T = 256, 128
EMBED, HIDDEN, VOCAB, FC_IN = 256, 512, 1004, 2048
NCORES = 8
BS = B // NCORES  # 32 batch rows per core
G4 = 4 * HIDDEN  # 2048 gate dim
VOCAB_TILES = 8  # 7*128 + 108
NT_CHUNKS = (T * BS) // 512  # 8 chunks of the (t,b) free dim in phase C

# device gate order: (i, f, o, g) so sigmoid(i,f,o) is partitions 0:96 and
# tanh(g) is partitions 96:128 in the gates PSUM tile (engine APs starting at
# partition 32/96 may span at most 32 partitions; starting at 0 up to 128).
GATE_PERM = (0, 1, 3, 2)  # source order is (i, f, g, o)

# ScalarE activation with input/output at different base partitions
# (partition shift). Falls back to DVE cross-quadrant copies if unsupported.
USE_ACT_SHIFT = True


def _bf16(x):
    return np.ascontiguousarray(x.astype(ml_dtypes.bfloat16))


def _f32(x):
    return np.ascontiguousarray(x.astype(np.float32))


def _reorder_gates_cols(w):
    """w[..., 2048] in (i,f,g,o) order -> (i,f,o,g) order."""
    blocks = [w[..., i * HIDDEN:(i + 1) * HIDDEN] for i in range(4)]
    return np.concatenate([blocks[i] for i in GATE_PERM], axis=-1)


def _part_fold(w, kdim):
    """[K, N] -> [128, K//128, N] so partition p holds rows {c*128+p}."""
    k, n = w.shape
    assert k == kdim and k % 128 == 0
    return np.ascontiguousarray(w.reshape(k // 128, 128, n).transpose(1, 0, 2))


def prepare_host(images, captions, W_fc, b_fc, W_att, b_att,
                 W_ih, b_ih, W_hh, b_hh, W_out, b_out):
    """Fold weights, reorder gates, build per-core input maps."""
    f64 = np.float64
    W_att_f = W_att[:, :EMBED].astype(f64)     # [256, 256]
    W_att_e = W_att[:, EMBED:].astype(f64)     # [256, 512]
    W_ih_f = W_ih[:, :EMBED].astype(f64)       # [2048, 256]
    W_ih_c = W_ih[:, EMBED:].astype(f64)       # [2048, 256]

    W1 = W_ih_f.T + W_att_f.T @ W_ih_c.T       # [256, 2048]
    b_eff = W_ih_c @ b_att.astype(f64) + b_ih.astype(f64) + b_hh.astype(f64)

    W1 = _reorder_gates_cols(W1)
    W_ihcT = _reorder_gates_cols(np.ascontiguousarray(W_ih_c.T))  # [256, 2048]
    b_eff = _reorder_gates_cols(b_eff[None, :])[0]
    W_hhT = _reorder_gates_cols(W_hh.T.astype(f64))  # [512, 2048]
    # sigma-trick: tanh(g) = 2*sigmoid(2g) - 1, so pre-scale the g-gate
    # (last 512 cols after reorder) by 2 and use one sigmoid over all gates.
    for w in (W1, W_ihcT, W_hhT):
        w[..., 3 * HIDDEN:] *= 2.0
    b_eff[3 * HIDDEN:] *= 2.0

    # shared (weight) tensors, partition-folded + bf16
    shared = {
        "wfc": _bf16(_part_fold(W_fc.T.astype(f64), FC_IN)),      # [128,16,256]
        "w1": _bf16(_part_fold(W1, EMBED)),                       # [128,2,2048]
        "watteT": _bf16(_part_fold(np.ascontiguousarray(W_att_e.T), HIDDEN)),
        "wihcT": _bf16(_part_fold(W_ihcT, EMBED)),                # [128,2,2048]
        "whh": _bf16(_part_fold(W_hhT, HIDDEN)),                  # [128,4,2048]
        "wout": _bf16(_part_fold(np.ascontiguousarray(W_out.T.astype(f64)),
                                 HIDDEN)),                        # [128,4,1004]
        "beff": _bf16(b_eff[None, :]),                            # [1,2048]
        "ones1": _bf16(np.ones((1, BS))),                         # [1,32]
        "bfc": _f32(b_fc.reshape(2, 128).T.copy()),               # [128,2]
        "bout": _f32(np.concatenate([b_out, np.zeros(1024 - VOCAB,
                                                     b_out.dtype)]
                                    ).reshape(8, 128).T.copy()),  # [128,8]
    }
    i32 = np.zeros((128, BS), np.float32)
    for j in range(4):
        i32[j * 32:(j + 1) * 32] = np.eye(32)
    shared["i32"] = _bf16(i32)
    shared["ident32"] = _bf16(np.eye(32))

    in_maps = []
    for c in range(NCORES):
        sl = slice(c * BS, (c + 1) * BS)
        img_t = images[sl].T.astype(np.float64)                   # [2048, 32]
        caps = captions[sl]                                       # [32,T,512]
        # capsT[t] = caps[:, t, :].T -> [T,128,4,32] -> pair timesteps:
        # [T/2, 128, 2, 4, 32] (512B contiguous per partition per DMA)
        capsT = caps.transpose(1, 2, 0).reshape(T, 4, 128, BS)
        capsT = capsT.transpose(0, 2, 1, 3).reshape(T // 2, 2, 128, 4, BS)
        capsT = capsT.transpose(0, 2, 1, 3, 4)
        m = dict(shared)
        m["imagesT"] = _bf16(_part_fold(img_t, FC_IN))            # [128,16,32]
        m["capsT"] = _bf16(capsT)                                 # [T/2,128,2,4,32]
        in_maps.append(m)
    return in_maps


def build_nc():
    nc = bacc.Bacc("TRN2", target_bir_lowering=False)

    # --- dram params (inputs) ---
    d_imagesT = nc.declare_dram_parameter("imagesT", [128, 16, BS], BF16, isOutput=False)
    d_capsT = nc.declare_dram_parameter("capsT", [T // 2, 128, 2, 4, BS], BF16, isOutput=False)
    d_wfc = nc.declare_dram_parameter("wfc", [128, 16, EMBED], BF16, isOutput=False)
    d_w1 = nc.declare_dram_parameter("w1", [128, 2, G4], BF16, isOutput=False)
    d_watteT = nc.declare_dram_parameter("watteT", [128, 4, EMBED], BF16, isOutput=False)
    d_wihcT = nc.declare_dram_parameter("wihcT", [128, 2, G4], BF16, isOutput=False)
    d_whh = nc.declare_dram_parameter("whh", [128, 4, G4], BF16, isOutput=False)
    d_wout = nc.declare_dram_parameter("wout", [128, 4, VOCAB], BF16, isOutput=False)
    d_beff = nc.declare_dram_parameter("beff", [1, G4], BF16, isOutput=False)
    d_ones1 = nc.declare_dram_parameter("ones1", [1, BS], BF16, isOutput=False)
    d_bfc = nc.declare_dram_parameter("bfc", [128, 2], F32, isOutput=False)
    d_bout = nc.declare_dram_parameter("bout", [128, 8], F32, isOutput=False)
    d_i32 = nc.declare_dram_parameter("i32", [128, BS], BF16, isOutput=False)
    d_ident32 = nc.declare_dram_parameter("ident32", [32, 32], BF16, isOutput=False)

    d_out = nc.declare_dram_parameter("outT", [VOCAB, T * BS], F32, isOutput=True)

    with tile.TileContext(nc) as tc:
        with (
            tc.tile_pool(name="weights", bufs=1) as wpool,
            tc.tile_pool(name="consts", bufs=1) as cpool,
            tc.tile_pool(name="caps", bufs=8) as cappool,
            tc.tile_pool(name="cell", bufs=4) as cellpool,
            tc.tile_pool(name="psg", bufs=2, space="PSUM") as psg,
            tc.tile_pool(name="pst", bufs=2, space="PSUM") as pst,
            tc.tile_pool(name="psc", bufs=2, space="PSUM") as psc,
            tc.tile_pool(name="outsb", bufs=6) as opool,
        ):
            # --- load weights/constants into SBUF ---
            sb_wfc = wpool.tile([128, 16, EMBED], BF16)
            sb_w1 = wpool.tile([128, 2, G4], BF16)
            sb_watteT = wpool.tile([128, 4, EMBED], BF16)
            sb_wihcT = wpool.tile([128, 2, G4], BF16)
            sb_whh = wpool.tile([128, 4, G4], BF16)
            sb_wout = wpool.tile([128, 4, VOCAB], BF16)
            sb_imgT = wpool.tile([128, 16, BS], BF16)
            sb_beff = cpool.tile([1, G4], BF16)
            sb_ones1 = cpool.tile([1, BS], BF16)
            sb_bfc = cpool.tile([128, 2], F32)
            sb_bout = cpool.tile([128, 8], F32)
            sb_i32 = cpool.tile([128, BS], BF16)
            sb_id32 = cpool.tile([32, 32], BF16)
            # ordered so tensors needed first (phase A, step 0) land first;
            # whh is not needed until t=1, wout until t=15
            for dst, src in [(sb_imgT, d_imagesT), (sb_wfc, d_wfc),
                             (sb_bfc, d_bfc), (sb_beff, d_beff),
                             (sb_ones1, d_ones1), (sb_i32, d_i32),
                             (sb_id32, d_ident32), (sb_w1, d_w1),
                             (sb_watteT, d_watteT), (sb_wihcT, d_wihcT),
                             (sb_whh, d_whh), (sb_wout, d_wout),
                             (sb_bout, d_bout)]:
                nc.sync.dma_start(out=dst[:], in_=src[:])

            # --- phase A: featsT [256, 32] = W_fc @ images_s.T  (2 M-tiles)
            sb_featsT = cpool.tile([128, 2, BS], BF16)
            for m in range(2):
                ps = psg.tile([128, HIDDEN], F32, tag="ps0")
                for k in range(16):
                    nc.tensor.matmul(
                        ps[:, :BS],
                        lhsT=sb_wfc[:, k, m * 128:(m + 1) * 128],
                        rhs=sb_imgT[:, k, :],
                        start=(k == 0), stop=(k == 15),
                    )
                # += b_fc broadcast along free dim (instructions carrying a
                # scalar/bias pointer have a single sync-wait slot in the ISA,
                # so use plain tensor_tensor with a stride-0 broadcast AP)
                nc.vector.tensor_add(
                    sb_featsT[:, m, :], ps[:, :BS],
                    sb_bfc[:, m:m + 1].broadcast_to((128, BS)),
                )

            # --- phase A2: gx0 [128(4gx32b), 512] = b_eff + feats @ W1
            sb_gx0 = cpool.tile([128, HIDDEN], BF16)
            ps0 = psg.tile([128, HIDDEN], F32, tag="ps0")
            for j in range(4):
                osl = ps0[32 * j:32 * (j + 1), :]
                nc.tensor.matmul(
                    osl, lhsT=sb_ones1[:, :],
                    rhs=sb_beff[:, j * HIDDEN:(j + 1) * HIDDEN],
                    start=True, stop=False, tile_position=(0, 32 * j),
                )
                for k in range(2):
                    nc.tensor.matmul(
                        osl, lhsT=sb_featsT[:, k, :],
                        rhs=sb_w1[:, k, j * HIDDEN:(j + 1) * HIDDEN],
                        start=False, stop=(k == 1), tile_position=(0, 32 * j),
                    )
            nc.vector.tensor_copy(sb_gx0[:], ps0[:])

            # --- phase B: the 128-step recurrence ---
            # hs_all holds hT for every step: [128, T, 4, 32] (hidden chunks
            # on partitions, (t, chunk, batch) on free)
            sb_hs = wpool.tile([128, T, 4, BS], BF16)
            sb_c = cpool.tile([64, HIDDEN], BF16)
            nc.vector.memset(sb_c[32:64, :], 0.0)

            out_units = [(n, m) for n in range(NT_CHUNKS)
                         for m in range(VOCAB_TILES)]

            def emit_out_unit(nchunk, m):
                tspan = slice(nchunk * (T // NT_CHUNKS),
                              (nchunk + 1) * (T // NT_CHUNKS))
                mv = 128 if m < 7 else VOCAB - 7 * 128
                msl = slice(m * 128, m * 128 + mv)
                ps_o = psc.tile([128, 512], F32, tag="ctx")
                for k in range(4):
                    nc.tensor.matmul(
                        ps_o[:mv, :],
                        lhsT=sb_wout[:, k, msl],
                        rhs=sb_hs[:, tspan, k, :],
                        start=(k == 0), stop=(k == 3),
                    )
                sb_o = opool.tile([128, 512], F32)
                nc.vector.tensor_add(
                    sb_o[:mv, :], ps_o[:mv, :],
                    sb_bout[:mv, m:m + 1].broadcast_to((mv, 512)),
                )
                dma_eng = nc.sync if (m + nchunk) % 2 == 0 else nc.scalar
                dma_eng.dma_start(
                    out=d_out[msl, nchunk * 512:(nchunk + 1) * 512],
                    in_=sb_o[:mv, :])

            hT_prev = None
            for t in range(T):
                if t % 2 == 0:
                    sb_cap2 = cappool.tile([128, 2, 4, BS], BF16)
                    nc.sync.dma_start(out=sb_cap2[:], in_=d_capsT[t // 2])
                if t % 2 == 0:
                    # stage 1 for the timestep pair: ctx_eT [256, 2, 32] =
                    # W_att_e @ emb_{t,t+1}.T (rank-256 factorization of the
                    # folded caption->gates product)
                    ps_cx = psc.tile([128, 2, 2, BS], F32, tag="ctx")
                    for m in range(2):
                        for k in range(4):
                            nc.tensor.matmul(
                                ps_cx[:, m, :, :],
                                lhsT=sb_watteT[:, k, m * 128:(m + 1) * 128],
                                rhs=sb_cap2[:, :, k, :],
                                start=(k == 0), stop=(k == 3),
                            )
                    sb_cx2 = cellpool.tile([128, 2, 2, BS], BF16, tag="cx")
                    nc.vector.tensor_copy(sb_cx2[:], ps_cx[:])
                sb_cx = sb_cx2[:, :, t % 2]

                # Split the gates by hidden-half so sigma/cell/transpose of
                # half A overlap the PE rounds and cell of half B, and the
                # next step's h-rounds k=0,1 can start as soon as half A's
                # hT chunks are stored.
                HH = HIDDEN // 2  # 256
                for x in range(2):
                    hsl = slice(x * HH, (x + 1) * HH)  # hidden half
                    ps = psg.tile([128, HH], F32, tag=f"ps{x}")
                    for j in range(4):
                        osl = ps[32 * j:32 * (j + 1), :]
                        gsl = slice(j * HIDDEN + x * HH,
                                    j * HIDDEN + (x + 1) * HH)
                        # seed: gates_x feats part + bias (identity matmul)
                        nc.tensor.matmul(
                            osl, lhsT=sb_i32[32 * j:32 * (j + 1), :],
                            rhs=sb_gx0[32 * j:32 * (j + 1), hsl],
                            start=True, stop=False,
                            tile_position=(32 * j, 32 * j),
                        )
                        # stage 2: ctx_e @ W_ih_c.T rounds (h-independent)
                        for k in range(2):
                            nc.tensor.matmul(
                                osl, lhsT=sb_cx[:, k, :],
                                rhs=sb_wihcT[:, k, gsl],
                                start=False, stop=(t == 0 and k == 1),
                                tile_position=(0, 32 * j),
                            )
                        # recurrent rounds
                        if t > 0:
                            for k in range(4):
                                nc.tensor.matmul(
                                    osl, lhsT=hT_prev[:, k, :],
                                    rhs=sb_whh[:, k, gsl],
                                    start=False, stop=(k == 3),
                                    tile_position=(0, 32 * j),
                                )

                    # cell (per hidden-half). DVE tensor_tensor requires both
                    # SBUF inputs at the same base partition, so align
                    # operands by quadrant: sigma(i,f,o) at quadrants 0,1,2;
                    # c at q1; tanh(g) lands at q0; tanh(c) at q2.
                    acts = cellpool.tile([128, HH], BF16, tag=f"acts{x}")
                    nc.scalar.activation(acts[:, :], ps[:, :],
                                         mybir.ActivationFunctionType.Sigmoid)
                    tgt = cellpool.tile([32, HH], BF16, tag=f"tgt{x}")
                    tct = cellpool.tile([96, HH], BF16, tag=f"tct{x}")
                    # tanh(g) = 2*sigmoid(2g) - 1 (g rows pre-scaled by 2);
                    # single-src op, cross-quadrant write q3 -> q0
                    nc.vector.tensor_scalar(
                        out=tgt[0:32, :], in0=acts[96:128, :],
                        scalar1=2.0, scalar2=-1.0,
                        op0=mybir.AluOpType.mult, op1=mybir.AluOpType.add,
                    )
                    t1 = cellpool.tile([64, HH], BF16, tag=f"t1{x}")
                    nc.vector.tensor_mul(t1[32:64, :], acts[0:32, :],
                                         tgt[0:32, :])
                    t2 = cellpool.tile([64, HH], BF16, tag=f"t2{x}")
                    nc.vector.tensor_mul(t2[32:64, :], acts[32:64, :],
                                         sb_c[32:64, hsl])
                    nc.vector.tensor_add(sb_c[32:64, hsl], t1[32:64, :],
                                         t2[32:64, :])
                    nc.scalar.activation(tct[64:96, :], sb_c[32:64, hsl],
                                         mybir.ActivationFunctionType.Tanh)
                    h = cellpool.tile([32, HH], BF16, tag=f"h{x}")
                    nc.vector.tensor_mul(h[:], acts[64:96, :], tct[64:96, :])

                    # transpose h half -> hT chunks 2x, 2x+1 of hs_all[t]
                    ps_t = pst.tile([128, 2, BS], BF16, tag="pst")
                    for k in range(2):
                        nc.tensor.transpose(ps_t[:, k, :],
                                            h[:, 128 * k:128 * (k + 1)],
                                            sb_id32[:, :])
                    nc.scalar.copy(sb_hs[:, t, 2 * x:2 * x + 2, :], ps_t[:])
                hT_prev = sb_hs[:, t]

                # phase C interleaved: the output chunk for timesteps
                # [16n, 16n+16) only needs hs rows written so far. Emit one
                # (chunk, m-tile) unit per step so its matmuls fill PE idle
                # slots in the chain without hogging psum slots.
                while out_units and out_units[0][0] * (T // NT_CHUNKS) + 15 <= t:
                    nchunk, m = out_units.pop(0)
                    emit_out_unit(nchunk, m)
                    if t < T - 1:
                        break
    nc.compile()
    return nc


_NC_CACHE = None


def kernel(**inputs) -> np.ndarray:
    global _NC_CACHE
    from concourse.bass_utils import run_bass_kernel_spmd

    in_maps = prepare_host(**inputs)
    if _NC_CACHE is None:
        _NC_CACHE = build_nc()
    nc = _NC_CACHE
    res = run_bass_kernel_spmd(nc, in_maps, list(range(NCORES)))
    outs = []
    for c in range(NCORES):
        o = res.results[c]["outT"]  # [1004, 4096] f32, free = (t, b)
        o = o.reshape(VOCAB, T, BS).transpose(2, 1, 0)  # [32, T, 1004]
        outs.append(o)
    return np.ascontiguousarray(np.concatenate(outs, axis=0).astype(np.float32))


if __name__ == "__main__":
    nc = build_nc()
    print("built ok")


# revision 34
# speedup vs baseline: 1.0189x; 1.0189x over previous
"""Trainium2 Bass kernel for nn_AttentionModel (image-caption LSTM with
pre-computable 'attention').

Math refactor (exact, weight-level):
    feats = images @ W_fc.T + b_fc
    ctx_t = [feats, emb_t] @ W_att.T + b_att
    x_t   = [feats, ctx_t]
    gates = x_t @ W_ih.T + b_ih + h @ W_hh.T + b_hh
Since ctx_t does not depend on h, fold the attention layer into effective
weights (host-side, fp32):
    gates_x_t = feats @ W1 + emb_t @ W2 + b_eff
      W1   = W_ih[:, :256].T + W_att[:, :256].T @ W_ih[:, 256:].T   [256, 2048]
      W2   = W_att[:, 256:].T @ W_ih[:, 256:].T                     [512, 2048]
      b_eff= W_ih[:, 256:] @ b_att + b_ih + b_hh                    [2048]
Only the h @ W_hh.T term is recurrent.

The caption->gates product is further rank-factored (W2 = W_att_e.T @
W_ih_c.T has rank 256): stage1 computes ctx_eT = W_att_e @ emb.T once per
timestep pair, stage2 multiplies by W_ih_c.T.

Device layout (per core, batch shard = 32):
  - gates PSUM, two banks per step (hidden halves of 256): each
    [128, 256] with partition groups of 32 = (i, f, o, g), free dim =
    hidden half.  Produced by col-tiled matmuls (tile_position=(0,32j)):
    gx0-seed via an identity matmul, 2 stage2 K-rounds (h-independent),
    4 W_hh K-rounds (recurrent).  Halving lets sigma/cell of half A
    overlap the PE rounds of half B and the next step's h-rounds.
  - LSTM cell on ScalarE sigmoid (one op for all gates; tanh(g) =
    2*sigmoid(2g)-1 with g pre-scaled by 2 on the host) + VectorE, bf16.
  - h transposed back to [hidden, batch] via PE transposes per half and
    stored into hs_all; the W_out projection is interleaved into the
    recurrence one (chunk, vocab-tile) unit per step so its matmuls fill
    PE idle slots.
"""

import sys

sys.path.insert(0, "/opt/trn_rl_repo")

import numpy as np
import ml_dtypes

import concourse.bass as bass
import concourse.tile as tile
from concourse import bacc, mybir

BF16 = mybir.dt.bfloat16
F32 = mybir.dt.float32

B, T = 256, 128
EMBED, HIDDEN, VOCAB, FC_IN = 256, 512, 1004, 2048
NCORES = 8
BS = B // NCORES  # 32 batch rows per core
G4 = 4 * HIDDEN  # 2048 gate dim
VOCAB_TILES = 8  # 7*128 + 108
NT_CHUNKS = (T * BS) // 512  # 8 chunks of the (t,b) free dim in phase C

# device gate order: (i, f, o, g) so sigmoid(i,f,o) is partitions 0:96 and
# tanh(g) is partitions 96:128 in the gates PSUM tile (engine APs starting at
# partition 32/96 may span at most 32 partitions; starting at 0 up to 128).
GATE_PERM = (0, 1, 3, 2)  # source order is (i, f, g, o)

# ScalarE activation with input/output at different base partitions
# (partition shift). Falls back to DVE cross-quadrant copies if unsupported.
USE_ACT_SHIFT = True


def _bf16(x):
    return np.ascontiguousarray(x.astype(ml_dtypes.bfloat16))


def _f32(x):
    return np.ascontiguousarray(x.astype(np.float32))


def _reorder_gates_cols(w):
    """w[..., 2048] in (i,f,g,o) order -> (i,f,o,g) order."""
    blocks = [w[..., i * HIDDEN:(i + 1) * HIDDEN] for i in range(4)]
    return np.concatenate([blocks[i] for i in GATE_PERM], axis=-1)


def _part_fold(w, kdim):
    """[K, N] -> [128, K//128, N] so partition p holds rows {c*128+p}."""
    k, n = w.shape
    assert k == kdim and k % 128 == 0
    return np.ascontiguousarray(w.reshape(k // 128, 128, n).transpose(1, 0, 2))


def prepare_host(images, captions, W_fc, b_fc, W_att, b_att,
                 W_ih, b_ih, W_hh, b_hh, W_out, b_out):
    """Fold weights, reorder gates, build per-core input maps."""
    f64 = np.float64
    W_att_f = W_att[:, :EMBED].astype(f64)     # [256, 256]
    W_att_e = W_att[:, EMBED:].astype(f64)     # [256, 512]
    W_ih_f = W_ih[:, :EMBED].astype(f64)       # [2048, 256]
    W_ih_c = W_ih[:, EMBED:].astype(f64)       # [2048, 256]

    W1 = W_ih_f.T + W_att_f.T @ W_ih_c.T       # [256, 2048]
    b_eff = W_ih_c @ b_att.astype(f64) + b_ih.astype(f64) + b_hh.astype(f64)

    W1 = _reorder_gates_cols(W1)
    W_ihcT = _reorder_gates_cols(np.ascontiguousarray(W_ih_c.T))  # [256, 2048]
    b_eff = _reorder_gates_cols(b_eff[None, :])[0]
    W_hhT = _reorder_gates_cols(W_hh.T.astype(f64))  # [512, 2048]
    # sigma-trick: tanh(g) = 2*sigmoid(2g) - 1, so pre-scale the g-gate
    # (last 512 cols after reorder) by 2 and use one sigmoid over all gates.
    for w in (W1, W_ihcT, W_hhT):
        w[..., 3 * HIDDEN:] *= 2.0
    b_eff[3 * HIDDEN:] *= 2.0

    # shared (weight) tensors, partition-folded + bf16
    shared = {
        "wfc": _bf16(_part_fold(W_fc.T.astype(f64), FC_IN)),      # [128,16,256]
        "w1": _bf16(_part_fold(W1, EMBED)),                       # [128,2,2048]
        "watteT": _bf16(_part_fold(np.ascontiguousarray(W_att_e.T), HIDDEN)),
        "wihcT": _bf16(_part_fold(W_ihcT, EMBED)),                # [128,2,2048]
        "whh": _bf16(_part_fold(W_hhT, HIDDEN)),                  # [128,4,2048]
        "wout": _bf16(_part_fold(np.ascontiguousarray(W_out.T.astype(f64)),
                                 HIDDEN)),                        # [128,4,1004]
        "beff": _bf16(b_eff[None, :]),                            # [1,2048]
        "ones1": _bf16(np.ones((1, BS))),                         # [1,32]
        "bfc": _f32(b_fc.reshape(2, 128).T.copy()),               # [128,2]
        "bout": _f32(np.concatenate([b_out, np.zeros(1024 - VOCAB,
                                                     b_out.dtype)]
                                    ).reshape(8, 128).T.copy()),  # [128,8]
    }
    i32 = np.zeros((128, BS), np.float32)
    for j in range(4):
        i32[j * 32:(j + 1) * 32] = np.eye(32)
    shared["i32"] = _bf16(i32)
    shared["ident32"] = _bf16(np.eye(32))

    in_maps = []
    for c in range(NCORES):
        sl = slice(c * BS, (c + 1) * BS)
        img_t = images[sl].T.astype(np.float64)                   # [2048, 32]
        caps = captions[sl]                                       # [32,T,512]
        # capsT[t] = caps[:, t, :].T -> [T,128,4,32] -> pair timesteps:
        # [T/2, 128, 2, 4, 32] (512B contiguous per partition per DMA)
        capsT = caps.transpose(1, 2, 0).reshape(T, 4, 128, BS)
        capsT = capsT.transpose(0, 2, 1, 3).reshape(T // 2, 2, 128, 4, BS)
        capsT = capsT.transpose(0, 2, 1, 3, 4)
        m = dict(shared)
        m["imagesT"] = _bf16(_part_fold(img_t, FC_IN))            # [128,16,32]
        m["capsT"] = _bf16(capsT)                                 # [T/2,128,2,4,32]
        in_maps.append(m)
    return in_maps


def build_nc():
    nc = bacc.Bacc("TRN2", target_bir_lowering=False)

    # --- dram params (inputs) ---
    d_imagesT = nc.declare_dram_parameter("imagesT", [128, 16, BS], BF16, isOutput=False)
    d_capsT = nc.declare_dram_parameter("capsT", [T // 2, 128, 2, 4, BS], BF16, isOutput=False)
    d_wfc = nc.declare_dram_parameter("wfc", [128, 16, EMBED], BF16, isOutput=False)
    d_w1 = nc.declare_dram_parameter("w1", [128, 2, G4], BF16, isOutput=False)
    d_watteT = nc.declare_dram_parameter("watteT", [128, 4, EMBED], BF16, isOutput=False)
    d_wihcT = nc.declare_dram_parameter("wihcT", [128, 2, G4], BF16, isOutput=False)
    d_whh = nc.declare_dram_parameter("whh", [128, 4, G4], BF16, isOutput=False)
    d_wout = nc.declare_dram_parameter("wout", [128, 4, VOCAB], BF16, isOutput=False)
    d_beff = nc.declare_dram_parameter("beff", [1, G4], BF16, isOutput=False)
    d_ones1 = nc.declare_dram_parameter("ones1", [1, BS], BF16, isOutput=False)
    d_bfc = nc.declare_dram_parameter("bfc", [128, 2], F32, isOutput=False)
    d_bout = nc.declare_dram_parameter("bout", [128, 8], F32, isOutput=False)
    d_i32 = nc.declare_dram_parameter("i32", [128, BS], BF16, isOutput=False)
    d_ident32 = nc.declare_dram_parameter("ident32", [32, 32], BF16, isOutput=False)

    d_out = nc.declare_dram_parameter("outT", [VOCAB, T * BS], F32, isOutput=True)

    with tile.TileContext(nc) as tc:
        with (
            tc.tile_pool(name="weights", bufs=1) as wpool,
            tc.tile_pool(name="consts", bufs=1) as cpool,
            tc.tile_pool(name="caps", bufs=8) as cappool,
            tc.tile_pool(name="cell", bufs=4) as cellpool,
            tc.tile_pool(name="psg", bufs=2, space="PSUM") as psg,
            tc.tile_pool(name="pst", bufs=2, space="PSUM") as pst,
            tc.tile_pool(name="psc", bufs=2, space="PSUM") as psc,
            tc.tile_pool(name="outsb", bufs=6) as opool,
        ):
            # --- load weights/constants into SBUF ---
            sb_wfc = wpool.tile([128, 16, EMBED], BF16)
            sb_w1 = wpool.tile([128, 2, G4], BF16)
            sb_watteT = wpool.tile([128, 4, EMBED], BF16)
            sb_wihcT = wpool.tile([128, 2, G4], BF16)
            sb_whh = wpool.tile([128, 4, G4], BF16)
            sb_wout = wpool.tile([128, 4, VOCAB], BF16)
            sb_imgT = wpool.tile([128, 16, BS], BF16)
            sb_beff = cpool.tile([1, G4], BF16)
            sb_ones1 = cpool.tile([1, BS], BF16)
            sb_bfc = cpool.tile([128, 2], F32)
            sb_bout = cpool.tile([128, 8], F32)
            sb_i32 = cpool.tile([128, BS], BF16)
            sb_id32 = cpool.tile([32, 32], BF16)
            # ordered so tensors needed first (phase A, step 0) land first;
            # whh is not needed until t=1, wout until t=15
            for dst, src in [(sb_imgT, d_imagesT), (sb_wfc, d_wfc),
                             (sb_bfc, d_bfc), (sb_beff, d_beff),
                             (sb_ones1, d_ones1), (sb_i32, d_i32),
                             (sb_id32, d_ident32), (sb_w1, d_w1),
                             (sb_watteT, d_watteT), (sb_wihcT, d_wihcT),
                             (sb_whh, d_whh), (sb_wout, d_wout),
                             (sb_bout, d_bout)]:
                nc.sync.dma_start(out=dst[:], in_=src[:])

            # --- phase A: featsT [256, 32] = W_fc @ images_s.T  (2 M-tiles)
            sb_featsT = cpool.tile([128, 2, BS], BF16)
            for m in range(2):
                ps = psg.tile([128, HIDDEN], F32, tag="ps0")
                for k in range(16):
                    nc.tensor.matmul(
                        ps[:, :BS],
                        lhsT=sb_wfc[:, k, m * 128:(m + 1) * 128],
                        rhs=sb_imgT[:, k, :],
                        start=(k == 0), stop=(k == 15),
                    )
                # += b_fc broadcast along free dim (instructions carrying a
                # scalar/bias pointer have a single sync-wait slot in the ISA,
                # so use plain tensor_tensor with a stride-0 broadcast AP)
                nc.vector.tensor_add(
                    sb_featsT[:, m, :], ps[:, :BS],
                    sb_bfc[:, m:m + 1].broadcast_to((128, BS)),
                )

            # --- phase A2: gx0 [128(4gx32b), 512] = b_eff + feats @ W1
            sb_gx0 = cpool.tile([128, HIDDEN], BF16)
            ps0 = psg.tile([128, HIDDEN], F32, tag="ps0")
            for j in range(4):
                osl = ps0[32 * j:32 * (j + 1), :]
                nc.tensor.matmul(
                    osl, lhsT=sb_ones1[:, :],
                    rhs=sb_beff[:, j * HIDDEN:(j + 1) * HIDDEN],
                    start=True, stop=False, tile_position=(0, 32 * j),
                )
                for k in range(2):
                    nc.tensor.matmul(
                        osl, lhsT=sb_featsT[:, k, :],
                        rhs=sb_w1[:, k, j * HIDDEN:(j + 1) * HIDDEN],
                        start=False, stop=(k == 1), tile_position=(0, 32 * j),
                    )
            nc.vector.tensor_copy(sb_gx0[:], ps0[:])

            # --- phase B: the 128-step recurrence ---
            # hs_all holds hT for every step: [128, T, 4, 32] (hidden chunks
            # on partitions, (t, chunk, batch) on free)
            sb_hs = wpool.tile([128, T, 4, BS], BF16)
            sb_c = cpool.tile([64, HIDDEN], BF16)
            nc.vector.memset(sb_c[32:64, :], 0.0)

            out_units = [(n, m) for n in range(NT_CHUNKS)
                         for m in range(VOCAB_TILES)]

            def emit_out_unit(nchunk, m):
                tspan = slice(nchunk * (T // NT_CHUNKS),
                              (nchunk + 1) * (T // NT_CHUNKS))
                mv = 128 if m < 7 else VOCAB - 7 * 128
                msl = slice(m * 128, m * 128 + mv)
                ps_o = psc.tile([128, 512], F32, tag="ctx")
                for k in range(4):
                    nc.tensor.matmul(
                        ps_o[:mv, :],
                        lhsT=sb_wout[:, k, msl],
                        rhs=sb_hs[:, tspan, k, :],
                        start=(k == 0), stop=(k == 3),
                    )
                sb_o = opool.tile([128, 512], F32)
                nc.vector.tensor_add(
                    sb_o[:mv, :], ps_o[:mv, :],
                    sb_bout[:mv, m:m + 1].broadcast_to((mv, 512)),
                )
                dma_eng = nc.sync if (m + nchunk) % 2 == 0 else nc.scalar
                dma_eng.dma_start(
                    out=d_out[msl, nchunk * 512:(nchunk + 1) * 512],
                    in_=sb_o[:mv, :])

            hT_prev = None
            for t in range(T):
                if t % 2 == 0:
                    sb_cap2 = cappool.tile([128, 2, 4, BS], BF16)
                    nc.sync.dma_start(out=sb_cap2[:], in_=d_capsT[t // 2])
                if t % 2 == 0:
                    # stage 1 for the timestep pair: ctx_eT [256, 2, 32] =
                    # W_att_e @ emb_{t,t+1}.T (rank-256 factorization of the
                    # folded caption->gates product)
                    ps_cx = psc.tile([128, 2, 2, BS], F32, tag="ctx")
                    for m in range(2):
                        for k in range(4):
                            nc.tensor.matmul(
                                ps_cx[:, m, :, :],
                                lhsT=sb_watteT[:, k, m * 128:(m + 1) * 128],
                                rhs=sb_cap2[:, :, k, :],
                                start=(k == 0), stop=(k == 3),
                            )
                    sb_cx2 = cellpool.tile([128, 2, 2, BS], BF16, tag="cx")
                    nc.vector.tensor_copy(sb_cx2[:], ps_cx[:])
                sb_cx = sb_cx2[:, :, t % 2]

                # Split the gates by hidden-half so sigma/cell/transpose of
                # half A overlap the PE rounds and cell of half B, and the
                # next step's h-rounds k=0,1 can start as soon as half A's
                # hT chunks are stored.
                HH = HIDDEN // 2  # 256
                for x in range(2):
                    hsl = slice(x * HH, (x + 1) * HH)  # hidden half
                    ps = psg.tile([128, HH], F32, tag=f"ps{x}")
                    for j in range(4):
                        osl = ps[32 * j:32 * (j + 1), :]
                        gsl = slice(j * HIDDEN + x * HH,
                                    j * HIDDEN + (x + 1) * HH)
                        # seed: gates_x feats part + bias (identity matmul)
                        nc.tensor.matmul(
                            osl, lhsT=sb_i32[32 * j:32 * (j + 1), :],
                            rhs=sb_gx0[32 * j:32 * (j + 1), hsl],
                            start=True, stop=False,
                            tile_position=(32 * j, 32 * j),
                        )
                        # stage 2: ctx_e @ W_ih_c.T rounds (h-independent)
                        for k in range(2):
                            nc.tensor.matmul(
                                osl, lhsT=sb_cx[:, k, :],
                                rhs=sb_wihcT[:, k, gsl],
                                start=False, stop=(t == 0 and k == 1),
                                tile_position=(0, 32 * j),
                            )
                        # recurrent rounds
                        if t > 0:
                            for k in range(4):
                                nc.tensor.matmul(
                                    osl, lhsT=hT_prev[:, k, :],
                                    rhs=sb_whh[:, k, gsl],
                                    start=False, stop=(k == 3),
                                    tile_position=(0, 32 * j),
                                )

                    # cell (per hidden-half). DVE tensor_tensor requires both
                    # SBUF inputs at the same base partition, so align
                    # operands by quadrant: sigma(i,f,o) at quadrants 0,1,2;
                    # c at q1; tanh(g) lands at q0; tanh(c) at q2.
                    acts = cellpool.tile([128, HH], BF16, tag=f"acts{x}")
                    nc.scalar.activation(acts[:, :], ps[:, :],
                                         mybir.ActivationFunctionType.Sigmoid)
                    tgt = cellpool.tile([32, HH], BF16, tag=f"tgt{x}")
                    tct = cellpool.tile([96, HH], BF16, tag=f"tct{x}")
                    # tanh(g) = 2*sigmoid(2g) - 1 (g rows pre-scaled by 2);
                    # single-src op, cross-quadrant write q3 -> q0
                    nc.vector.tensor_scalar(
                        out=tgt[0:32, :], in0=acts[96:128, :],
                        scalar1=2.0, scalar2=-1.0,
                        op0=mybir.AluOpType.mult, op1=mybir.AluOpType.add,
                    )
                    t1 = cellpool.tile([64, HH], BF16, tag=f"t1{x}")
                    nc.vector.tensor_mul(t1[32:64, :], acts[0:32, :],
                                         tgt[0:32, :])
                    t2 = cellpool.tile([64, HH], BF16, tag=f"t2{x}")
                    nc.vector.tensor_mul(t2[32:64, :], acts[32:64, :],
                                         sb_c[32:64, hsl])
                    nc.vector.tensor_add(sb_c[32:64, hsl], t1[32:64, :],
                                         t2[32:64, :])
                    nc.scalar.activation(tct[64:96, :], sb_c[32:64, hsl],
                                         mybir.ActivationFunctionType.Tanh)
                    h = cellpool.tile([32, HH], BF16, tag=f"h{x}")
                    nc.vector.tensor_mul(h[:], acts[64:96, :], tct[64:96, :])

                    # transpose h half -> hT chunks 2x, 2x+1 of hs_all[t]
                    ps_t = pst.tile([128, 2, BS], BF16, tag="pst")
                    for k in range(2):
                        nc.tensor.transpose(ps_t[:, k, :],
                                            h[:, 128 * k:128 * (k + 1)],
                                            sb_id32[:, :])
                    nc.vector.tensor_copy(sb_hs[:, t, 2 * x:2 * x + 2, :], ps_t[:])
                hT_prev = sb_hs[:, t]

                # phase C interleaved: the output chunk for timesteps
                # [16n, 16n+16) only needs hs rows written so far. Emit one
                # (chunk, m-tile) unit per step so its matmuls fill PE idle
                # slots in the chain without hogging psum slots.
                while out_units and out_units[0][0] * (T // NT_CHUNKS) + 15 <= t:
                    nchunk, m = out_units.pop(0)
                    emit_out_unit(nchunk, m)
                    if t < T - 1:
                        break
    nc.compile()
    return nc


_NC_CACHE = None


def kernel(**inputs) -> np.ndarray:
    global _NC_CACHE
    from concourse.bass_utils import run_bass_kernel_spmd

    in_maps = prepare_host(**inputs)
    if _NC_CACHE is None:
        _NC_CACHE = build_nc()
    nc = _NC_CACHE
    res = run_bass_kernel_spmd(nc, in_maps, list(range(NCORES)))
    outs = []
    for c in range(NCORES):
        o = res.results[c]["outT"]  # [1004, 4096] f32, free = (t, b)
        o = o.reshape(VOCAB, T, BS).transpose(2, 1, 0)  # [32, T, 1004]
        outs.append(o)
    return np.ascontiguousarray(np.concatenate(outs, axis=0).astype(np.float32))


if __name__ == "__main__":
    nc = build_nc()
    print("built ok")


# revision 35
# speedup vs baseline: 1.0352x; 1.0160x over previous
"""Trainium2 Bass kernel for nn_AttentionModel (image-caption LSTM with
pre-computable 'attention').

Math refactor (exact, weight-level):
    feats = images @ W_fc.T + b_fc
    ctx_t = [feats, emb_t] @ W_att.T + b_att
    x_t   = [feats, ctx_t]
    gates = x_t @ W_ih.T + b_ih + h @ W_hh.T + b_hh
Since ctx_t does not depend on h, fold the attention layer into effective
weights (host-side, fp32):
    gates_x_t = feats @ W1 + emb_t @ W2 + b_eff
      W1   = W_ih[:, :256].T + W_att[:, :256].T @ W_ih[:, 256:].T   [256, 2048]
      W2   = W_att[:, 256:].T @ W_ih[:, 256:].T                     [512, 2048]
      b_eff= W_ih[:, 256:] @ b_att + b_ih + b_hh                    [2048]
Only the h @ W_hh.T term is recurrent.

The caption->gates product is further rank-factored (W2 = W_att_e.T @
W_ih_c.T has rank 256): stage1 computes ctx_eT = W_att_e @ emb.T once per
timestep pair, stage2 multiplies by W_ih_c.T.

Device layout (per core, batch shard = 32):
  - gates PSUM, two banks per step (hidden halves of 256): each
    [128, 256] with partition groups of 32 = (i, f, o, g), free dim =
    hidden half.  Produced by col-tiled matmuls (tile_position=(0,32j)):
    gx0-seed via an identity matmul, 2 stage2 K-rounds (h-independent),
    4 W_hh K-rounds (recurrent).  Halving lets sigma/cell of half A
    overlap the PE rounds of half B and the next step's h-rounds.
  - LSTM cell on ScalarE sigmoid (one op for all gates; tanh(g) =
    2*sigmoid(2g)-1 with g pre-scaled by 2 on the host) + VectorE, bf16.
  - h transposed back to [hidden, batch] via PE transposes per half and
    stored into hs_all; the W_out projection is interleaved into the
    recurrence one (chunk, vocab-tile) unit per step so its matmuls fill
    PE idle slots.
"""

import sys

sys.path.insert(0, "/opt/trn_rl_repo")

import numpy as np
import ml_dtypes

import concourse.bass as bass
import concourse.tile as tile
from concourse import bacc, mybir

BF16 = mybir.dt.bfloat16
F32 = mybir.dt.float32

B, T = 256, 128
EMBED, HIDDEN, VOCAB, FC_IN = 256, 512, 1004, 2048
NCORES = 8
BS = B // NCORES  # 32 batch rows per core
G4 = 4 * HIDDEN  # 2048 gate dim
VOCAB_TILES = 8  # 7*128 + 108
NT_CHUNKS = (T * BS) // 512  # 8 chunks of the (t,b) free dim in phase C

# device gate order: (i, f, o, g) so sigmoid(i,f,o) is partitions 0:96 and
# tanh(g) is partitions 96:128 in the gates PSUM tile (engine APs starting at
# partition 32/96 may span at most 32 partitions; starting at 0 up to 128).
GATE_PERM = (0, 1, 3, 2)  # source order is (i, f, g, o)

# ScalarE activation with input/output at different base partitions
# (partition shift). Falls back to DVE cross-quadrant copies if unsupported.
USE_ACT_SHIFT = True


def _bf16(x):
    return np.ascontiguousarray(x.astype(ml_dtypes.bfloat16))


def _f32(x):
    return np.ascontiguousarray(x.astype(np.float32))


def _reorder_gates_cols(w):
    """w[..., 2048] in (i,f,g,o) order -> (i,f,o,g) order."""
    blocks = [w[..., i * HIDDEN:(i + 1) * HIDDEN] for i in range(4)]
    return np.concatenate([blocks[i] for i in GATE_PERM], axis=-1)


def _part_fold(w, kdim):
    """[K, N] -> [128, K//128, N] so partition p holds rows {c*128+p}."""
    k, n = w.shape
    assert k == kdim and k % 128 == 0
    return np.ascontiguousarray(w.reshape(k // 128, 128, n).transpose(1, 0, 2))


def prepare_host(images, captions, W_fc, b_fc, W_att, b_att,
                 W_ih, b_ih, W_hh, b_hh, W_out, b_out):
    """Fold weights, reorder gates, build per-core input maps."""
    f64 = np.float64
    W_att_f = W_att[:, :EMBED].astype(f64)     # [256, 256]
    W_att_e = W_att[:, EMBED:].astype(f64)     # [256, 512]
    W_ih_f = W_ih[:, :EMBED].astype(f64)       # [2048, 256]
    W_ih_c = W_ih[:, EMBED:].astype(f64)       # [2048, 256]

    W1 = W_ih_f.T + W_att_f.T @ W_ih_c.T       # [256, 2048]
    b_eff = W_ih_c @ b_att.astype(f64) + b_ih.astype(f64) + b_hh.astype(f64)

    W1 = _reorder_gates_cols(W1)
    W_ihcT = _reorder_gates_cols(np.ascontiguousarray(W_ih_c.T))  # [256, 2048]
    b_eff = _reorder_gates_cols(b_eff[None, :])[0]
    W_hhT = _reorder_gates_cols(W_hh.T.astype(f64))  # [512, 2048]
    # sigma-trick: tanh(g) = 2*sigmoid(2g) - 1, so pre-scale the g-gate
    # (last 512 cols after reorder) by 2 and use one sigmoid over all gates.
    for w in (W1, W_ihcT, W_hhT):
        w[..., 3 * HIDDEN:] *= 2.0
    b_eff[3 * HIDDEN:] *= 2.0

    # shared (weight) tensors, partition-folded + bf16
    shared = {
        "wfc": _bf16(_part_fold(W_fc.T.astype(f64), FC_IN)),      # [128,16,256]
        "w1": _bf16(_part_fold(W1, EMBED)),                       # [128,2,2048]
        "watteT": _bf16(_part_fold(np.ascontiguousarray(W_att_e.T), HIDDEN)),
        "wihcT": _bf16(_part_fold(W_ihcT, EMBED)),                # [128,2,2048]
        "whh": _bf16(_part_fold(W_hhT, HIDDEN)),                  # [128,4,2048]
        "wout": _bf16(_part_fold(np.ascontiguousarray(W_out.T.astype(f64)),
                                 HIDDEN)),                        # [128,4,1004]
        "beff": _bf16(b_eff[None, :]),                            # [1,2048]
        "ones1": _bf16(np.ones((1, BS))),                         # [1,32]
        "bfc": _f32(b_fc.reshape(2, 128).T.copy()),               # [128,2]
        "bout": _f32(np.concatenate([b_out, np.zeros(1024 - VOCAB,
                                                     b_out.dtype)]
                                    ).reshape(8, 128).T.copy()),  # [128,8]
    }
    i32 = np.zeros((128, BS), np.float32)
    for j in range(4):
        i32[j * 32:(j + 1) * 32] = np.eye(32)
    shared["i32"] = _bf16(i32)
    shared["ident32"] = _bf16(np.eye(32))

    in_maps = []
    for c in range(NCORES):
        sl = slice(c * BS, (c + 1) * BS)
        img_t = images[sl].T.astype(np.float64)                   # [2048, 32]
        caps = captions[sl]                                       # [32,T,512]
        # capsT[t] = caps[:, t, :].T -> [T,128,4,32] -> pair timesteps:
        # [T/2, 128, 2, 4, 32] (512B contiguous per partition per DMA)
        capsT = caps.transpose(1, 2, 0).reshape(T, 4, 128, BS)
        capsT = capsT.transpose(0, 2, 1, 3).reshape(T // 2, 2, 128, 4, BS)
        capsT = capsT.transpose(0, 2, 1, 3, 4)
        m = dict(shared)
        m["imagesT"] = _bf16(_part_fold(img_t, FC_IN))            # [128,16,32]
        m["capsT"] = _bf16(capsT)                                 # [T/2,128,2,4,32]
        in_maps.append(m)
    return in_maps


def build_nc():
    nc = bacc.Bacc("TRN2", target_bir_lowering=False)

    # --- dram params (inputs) ---
    d_imagesT = nc.declare_dram_parameter("imagesT", [128, 16, BS], BF16, isOutput=False)
    d_capsT = nc.declare_dram_parameter("capsT", [T // 2, 128, 2, 4, BS], BF16, isOutput=False)
    d_wfc = nc.declare_dram_parameter("wfc", [128, 16, EMBED], BF16, isOutput=False)
    d_w1 = nc.declare_dram_parameter("w1", [128, 2, G4], BF16, isOutput=False)
    d_watteT = nc.declare_dram_parameter("watteT", [128, 4, EMBED], BF16, isOutput=False)
    d_wihcT = nc.declare_dram_parameter("wihcT", [128, 2, G4], BF16, isOutput=False)
    d_whh = nc.declare_dram_parameter("whh", [128, 4, G4], BF16, isOutput=False)
    d_wout = nc.declare_dram_parameter("wout", [128, 4, VOCAB], BF16, isOutput=False)
    d_beff = nc.declare_dram_parameter("beff", [1, G4], BF16, isOutput=False)
    d_ones1 = nc.declare_dram_parameter("ones1", [1, BS], BF16, isOutput=False)
    d_bfc = nc.declare_dram_parameter("bfc", [128, 2], F32, isOutput=False)
    d_bout = nc.declare_dram_parameter("bout", [128, 8], F32, isOutput=False)
    d_i32 = nc.declare_dram_parameter("i32", [128, BS], BF16, isOutput=False)
    d_ident32 = nc.declare_dram_parameter("ident32", [32, 32], BF16, isOutput=False)

    d_out = nc.declare_dram_parameter("outT", [VOCAB, T * BS], F32, isOutput=True)

    with tile.TileContext(nc) as tc:
        with (
            tc.tile_pool(name="weights", bufs=1) as wpool,
            tc.tile_pool(name="consts", bufs=1) as cpool,
            tc.tile_pool(name="caps", bufs=8) as cappool,
            tc.tile_pool(name="cell", bufs=4) as cellpool,
            tc.tile_pool(name="psg", bufs=2, space="PSUM") as psg,
            tc.tile_pool(name="pst", bufs=2, space="PSUM") as pst,
            tc.tile_pool(name="psc", bufs=2, space="PSUM") as psc,
            tc.tile_pool(name="outsb", bufs=6) as opool,
        ):
            # --- load weights/constants into SBUF ---
            sb_wfc = wpool.tile([128, 16, EMBED], BF16)
            sb_w1 = wpool.tile([128, 2, G4], BF16)
            sb_watteT = wpool.tile([128, 4, EMBED], BF16)
            sb_wihcT = wpool.tile([128, 2, G4], BF16)
            sb_whh = wpool.tile([128, 4, G4], BF16)
            sb_wout = wpool.tile([128, 4, VOCAB], BF16)
            sb_imgT = wpool.tile([128, 16, BS], BF16)
            sb_beff = cpool.tile([1, G4], BF16)
            sb_ones1 = cpool.tile([1, BS], BF16)
            sb_bfc = cpool.tile([128, 2], F32)
            sb_bout = cpool.tile([128, 8], F32)
            sb_i32 = cpool.tile([128, BS], BF16)
            sb_id32 = cpool.tile([32, 32], BF16)
            # ordered so tensors needed first (phase A, step 0) land first;
            # whh is not needed until t=1, wout until t=15
            for dst, src in [(sb_imgT, d_imagesT), (sb_wfc, d_wfc),
                             (sb_bfc, d_bfc), (sb_beff, d_beff),
                             (sb_ones1, d_ones1), (sb_i32, d_i32),
                             (sb_id32, d_ident32), (sb_w1, d_w1),
                             (sb_watteT, d_watteT), (sb_wihcT, d_wihcT),
                             (sb_whh, d_whh), (sb_wout, d_wout),
                             (sb_bout, d_bout)]:
                nc.sync.dma_start(out=dst[:], in_=src[:])

            # --- phase A: featsT [256, 32] = W_fc @ images_s.T  (2 M-tiles)
            sb_featsT = cpool.tile([128, 2, BS], BF16)
            for m in range(2):
                ps = psg.tile([128, HIDDEN], F32, tag="ps0")
                for k in range(16):
                    nc.tensor.matmul(
                        ps[:, :BS],
                        lhsT=sb_wfc[:, k, m * 128:(m + 1) * 128],
                        rhs=sb_imgT[:, k, :],
                        start=(k == 0), stop=(k == 15),
                    )
                # += b_fc broadcast along free dim (instructions carrying a
                # scalar/bias pointer have a single sync-wait slot in the ISA,
                # so use plain tensor_tensor with a stride-0 broadcast AP)
                nc.vector.tensor_add(
                    sb_featsT[:, m, :], ps[:, :BS],
                    sb_bfc[:, m:m + 1].broadcast_to((128, BS)),
                )

            # --- phase A2: gx0 [128(4gx32b), 512] = b_eff + feats @ W1
            sb_gx0 = cpool.tile([128, HIDDEN], BF16)
            ps0 = psg.tile([128, HIDDEN], F32, tag="ps0")
            for j in range(4):
                osl = ps0[32 * j:32 * (j + 1), :]
                nc.tensor.matmul(
                    osl, lhsT=sb_ones1[:, :],
                    rhs=sb_beff[:, j * HIDDEN:(j + 1) * HIDDEN],
                    start=True, stop=False, tile_position=(0, 32 * j),
                )
                for k in range(2):
                    nc.tensor.matmul(
                        osl, lhsT=sb_featsT[:, k, :],
                        rhs=sb_w1[:, k, j * HIDDEN:(j + 1) * HIDDEN],
                        start=False, stop=(k == 1), tile_position=(0, 32 * j),
                    )
            nc.vector.tensor_copy(sb_gx0[:], ps0[:])

            # --- phase B: the 128-step recurrence ---
            # hs_all holds hT for every step: [128, T, 4, 32] (hidden chunks
            # on partitions, (t, chunk, batch) on free)
            sb_hs = wpool.tile([128, T, 4, BS], BF16)
            sb_c = cpool.tile([64, HIDDEN], BF16)
            nc.vector.memset(sb_c[32:64, :], 0.0)

            out_units = [(n, m) for n in range(NT_CHUNKS)
                         for m in range(VOCAB_TILES)]

            def emit_out_unit(nchunk, m):
                tspan = slice(nchunk * (T // NT_CHUNKS),
                              (nchunk + 1) * (T // NT_CHUNKS))
                mv = 128 if m < 7 else VOCAB - 7 * 128
                msl = slice(m * 128, m * 128 + mv)
                ps_o = psc.tile([128, 512], F32, tag="ctx")
                for k in range(4):
                    nc.tensor.matmul(
                        ps_o[:mv, :],
                        lhsT=sb_wout[:, k, msl],
                        rhs=sb_hs[:, tspan, k, :],
                        start=(k == 0), stop=(k == 3),
                    )
                sb_o = opool.tile([128, 512], F32)
                # ScalarE Identity+bias: b_out is per-partition here (vocab
                # on partitions); keeps the eviction off DVE, which carries
                # the chain's cell ops (Bacc legalizes the extra sync waits)
                nc.scalar.activation(
                    sb_o[:mv, :], ps_o[:mv, :],
                    mybir.ActivationFunctionType.Identity,
                    bias=sb_bout[:mv, m:m + 1],
                )
                dma_eng = nc.sync if (m + nchunk) % 2 == 0 else nc.scalar
                dma_eng.dma_start(
                    out=d_out[msl, nchunk * 512:(nchunk + 1) * 512],
                    in_=sb_o[:mv, :])

            hT_prev = None
            for t in range(T):
                if t % 2 == 0:
                    sb_cap2 = cappool.tile([128, 2, 4, BS], BF16)
                    nc.sync.dma_start(out=sb_cap2[:], in_=d_capsT[t // 2])
                if t % 2 == 0:
                    # stage 1 for the timestep pair: ctx_eT [256, 2, 32] =
                    # W_att_e @ emb_{t,t+1}.T (rank-256 factorization of the
                    # folded caption->gates product)
                    ps_cx = psc.tile([128, 2, 2, BS], F32, tag="ctx")
                    for m in range(2):
                        for k in range(4):
                            nc.tensor.matmul(
                                ps_cx[:, m, :, :],
                                lhsT=sb_watteT[:, k, m * 128:(m + 1) * 128],
                                rhs=sb_cap2[:, :, k, :],
                                start=(k == 0), stop=(k == 3),
                            )
                    sb_cx2 = cellpool.tile([128, 2, 2, BS], BF16, tag="cx")
                    nc.scalar.copy(sb_cx2[:], ps_cx[:])
                sb_cx = sb_cx2[:, :, t % 2]

                # Split the gates by hidden-half so sigma/cell/transpose of
                # half A overlap the PE rounds and cell of half B, and the
                # next step's h-rounds k=0,1 can start as soon as half A's
                # hT chunks are stored.
                HH = HIDDEN // 2  # 256
                for x in range(2):
                    hsl = slice(x * HH, (x + 1) * HH)  # hidden half
                    ps = psg.tile([128, HH], F32, tag=f"ps{x}")
                    for j in range(4):
                        osl = ps[32 * j:32 * (j + 1), :]
                        gsl = slice(j * HIDDEN + x * HH,
                                    j * HIDDEN + (x + 1) * HH)
                        # seed: gates_x feats part + bias (identity matmul)
                        nc.tensor.matmul(
                            osl, lhsT=sb_i32[32 * j:32 * (j + 1), :],
                            rhs=sb_gx0[32 * j:32 * (j + 1), hsl],
                            start=True, stop=False,
                            tile_position=(32 * j, 32 * j),
                        )
                        # stage 2: ctx_e @ W_ih_c.T rounds (h-independent)
                        for k in range(2):
                            nc.tensor.matmul(
                                osl, lhsT=sb_cx[:, k, :],
                                rhs=sb_wihcT[:, k, gsl],
                                start=False, stop=(t == 0 and k == 1),
                                tile_position=(0, 32 * j),
                            )
                        # recurrent rounds
                        if t > 0:
                            for k in range(4):
                                nc.tensor.matmul(
                                    osl, lhsT=hT_prev[:, k, :],
                                    rhs=sb_whh[:, k, gsl],
                                    start=False, stop=(k == 3),
                                    tile_position=(0, 32 * j),
                                )

                    # cell (per hidden-half). DVE tensor_tensor requires both
                    # SBUF inputs at the same base partition, so align
                    # operands by quadrant: sigma(i,f,o) at quadrants 0,1,2;
                    # c at q1; tanh(g) lands at q0; tanh(c) at q2.
                    acts = cellpool.tile([128, HH], BF16, tag=f"acts{x}")
                    nc.scalar.activation(acts[:, :], ps[:, :],
                                         mybir.ActivationFunctionType.Sigmoid)
                    tgt = cellpool.tile([32, HH], BF16, tag=f"tgt{x}")
                    tct = cellpool.tile([96, HH], BF16, tag=f"tct{x}")
                    # tanh(g) = 2*sigmoid(2g) - 1 (g rows pre-scaled by 2);
                    # single-src op, cross-quadrant write q3 -> q0
                    nc.vector.tensor_scalar(
                        out=tgt[0:32, :], in0=acts[96:128, :],
                        scalar1=2.0, scalar2=-1.0,
                        op0=mybir.AluOpType.mult, op1=mybir.AluOpType.add,
                    )
                    t1 = cellpool.tile([64, HH], BF16, tag=f"t1{x}")
                    nc.vector.tensor_mul(t1[32:64, :], acts[0:32, :],
                                         tgt[0:32, :])
                    t2 = cellpool.tile([64, HH], BF16, tag=f"t2{x}")
                    nc.vector.tensor_mul(t2[32:64, :], acts[32:64, :],
                                         sb_c[32:64, hsl])
                    nc.vector.tensor_add(sb_c[32:64, hsl], t1[32:64, :],
                                         t2[32:64, :])
                    nc.scalar.activation(tct[64:96, :], sb_c[32:64, hsl],
                                         mybir.ActivationFunctionType.Tanh)
                    h = cellpool.tile([32, HH], BF16, tag=f"h{x}")
                    nc.vector.tensor_mul(h[:], acts[64:96, :], tct[64:96, :])

                    # transpose h half -> hT chunks 2x, 2x+1 of hs_all[t]
                    ps_t = pst.tile([128, 2, BS], BF16, tag="pst")
                    for k in range(2):
                        nc.tensor.transpose(ps_t[:, k, :],
                                            h[:, 128 * k:128 * (k + 1)],
                                            sb_id32[:, :])
                    nc.vector.tensor_copy(sb_hs[:, t, 2 * x:2 * x + 2, :], ps_t[:])
                hT_prev = sb_hs[:, t]

                # phase C interleaved: the output chunk for timesteps
                # [16n, 16n+16) only needs hs rows written so far. Emit one
                # (chunk, m-tile) unit per step so its matmuls fill PE idle
                # slots in the chain without hogging psum slots.
                while out_units and out_units[0][0] * (T // NT_CHUNKS) + 15 <= t:
                    nchunk, m = out_units.pop(0)
                    emit_out_unit(nchunk, m)
                    if t < T - 1:
                        break
    nc.compile()
    return nc


_NC_CACHE = None


def kernel(**inputs) -> np.ndarray:
    global _NC_CACHE
    from concourse.bass_utils import run_bass_kernel_spmd

    in_maps = prepare_host(**inputs)
    if _NC_CACHE is None:
        _NC_CACHE = build_nc()
    nc = _NC_CACHE
    res = run_bass_kernel_spmd(nc, in_maps, list(range(NCORES)))
    outs = []
    for c in range(NCORES):
        o = res.results[c]["outT"]  # [1004, 4096] f32, free = (t, b)
        o = o.reshape(VOCAB, T, BS).transpose(2, 1, 0)  # [32, T, 1004]
        outs.append(o)
    return np.ascontiguousarray(np.concatenate(outs, axis=0).astype(np.float32))


if __name__ == "__main__":
    nc = build_nc()
    print("built ok")


# revision 43
# speedup vs baseline: 1.0394x; 1.0041x over previous
"""Trainium2 Bass kernel for nn_AttentionModel (image-caption LSTM with
pre-computable 'attention').

Math refactor (exact, weight-level):
    feats = images @ W_fc.T + b_fc
    ctx_t = [feats, emb_t] @ W_att.T + b_att
    x_t   = [feats, ctx_t]
    gates = x_t @ W_ih.T + b_ih + h @ W_hh.T + b_hh
Since ctx_t does not depend on h, fold the attention layer into effective
weights (host-side, fp32):
    gates_x_t = feats @ W1 + emb_t @ W2 + b_eff
      W1   = W_ih[:, :256].T + W_att[:, :256].T @ W_ih[:, 256:].T   [256, 2048]
      W2   = W_att[:, 256:].T @ W_ih[:, 256:].T                     [512, 2048]
      b_eff= W_ih[:, 256:] @ b_att + b_ih + b_hh                    [2048]
Only the h @ W_hh.T term is recurrent.

The caption->gates product is further rank-factored (W2 = W_att_e.T @
W_ih_c.T has rank 256): stage1 computes ctx_eT = W_att_e @ emb.T once per
timestep pair, stage2 multiplies by W_ih_c.T.

Device layout (per core, batch shard = 32):
  - gates PSUM, two banks per step (hidden halves of 256): each
    [128, 256] with partition groups of 32 = (i, f, o, g), free dim =
    hidden half.  Produced by col-tiled matmuls (tile_position=(0,32j)):
    gx0-seed via an identity matmul, 2 stage2 K-rounds (h-independent),
    4 W_hh K-rounds (recurrent).  Halving lets sigma/cell of half A
    overlap the PE rounds of half B and the next step's h-rounds.
  - LSTM cell on ScalarE sigmoid (one op for all gates; tanh(g) =
    2*sigmoid(2g)-1 with g pre-scaled by 2 on the host) + VectorE, bf16.
  - h transposed back to [hidden, batch] via PE transposes per half and
    stored into hs_all; the W_out projection is interleaved into the
    recurrence one (chunk, vocab-tile) unit per step so its matmuls fill
    PE idle slots.
"""

import sys

sys.path.insert(0, "/opt/trn_rl_repo")

import numpy as np
import ml_dtypes

import concourse.bass as bass
import concourse.tile as tile
from concourse import bacc, mybir

BF16 = mybir.dt.bfloat16
F32 = mybir.dt.float32

B, T = 256, 128
EMBED, HIDDEN, VOCAB, FC_IN = 256, 512, 1004, 2048
NCORES = 8
BS = B // NCORES  # 32 batch rows per core
G4 = 4 * HIDDEN  # 2048 gate dim
VOCAB_TILES = 8  # 7*128 + 108
NT_CHUNKS = (T * BS) // 512  # 8 chunks of the (t,b) free dim in phase C

# device gate order: (i, f, o, g) so sigmoid(i,f,o) is partitions 0:96 and
# tanh(g) is partitions 96:128 in the gates PSUM tile (engine APs starting at
# partition 32/96 may span at most 32 partitions; starting at 0 up to 128).
GATE_PERM = (0, 1, 3, 2)  # source order is (i, f, g, o)

# ScalarE activation with input/output at different base partitions
# (partition shift). Falls back to DVE cross-quadrant copies if unsupported.
USE_ACT_SHIFT = True


def _bf16(x):
    return np.ascontiguousarray(x.astype(ml_dtypes.bfloat16))


def _f32(x):
    return np.ascontiguousarray(x.astype(np.float32))


def _reorder_gates_cols(w):
    """w[..., 2048] in (i,f,g,o) order -> (i,f,o,g) order."""
    blocks = [w[..., i * HIDDEN:(i + 1) * HIDDEN] for i in range(4)]
    return np.concatenate([blocks[i] for i in GATE_PERM], axis=-1)


def _part_fold(w, kdim):
    """[K, N] -> [128, K//128, N] so partition p holds rows {c*128+p}."""
    k, n = w.shape
    assert k == kdim and k % 128 == 0
    return np.ascontiguousarray(w.reshape(k // 128, 128, n).transpose(1, 0, 2))


def prepare_host(images, captions, W_fc, b_fc, W_att, b_att,
                 W_ih, b_ih, W_hh, b_hh, W_out, b_out):
    """Fold weights, reorder gates, build per-core input maps."""
    f64 = np.float64
    W_att_f = W_att[:, :EMBED].astype(f64)     # [256, 256]
    W_att_e = W_att[:, EMBED:].astype(f64)     # [256, 512]
    W_ih_f = W_ih[:, :EMBED].astype(f64)       # [2048, 256]
    W_ih_c = W_ih[:, EMBED:].astype(f64)       # [2048, 256]

    W1 = W_ih_f.T + W_att_f.T @ W_ih_c.T       # [256, 2048]
    b_eff = W_ih_c @ b_att.astype(f64) + b_ih.astype(f64) + b_hh.astype(f64)

    W1 = _reorder_gates_cols(W1)
    W_ihcT = _reorder_gates_cols(np.ascontiguousarray(W_ih_c.T))  # [256, 2048]
    b_eff = _reorder_gates_cols(b_eff[None, :])[0]
    W_hhT = _reorder_gates_cols(W_hh.T.astype(f64))  # [512, 2048]
    # sigma-trick: tanh(g) = 2*sigmoid(2g) - 1, so pre-scale the g-gate
    # (last 512 cols after reorder) by 2 and use one sigmoid over all gates.
    for w in (W1, W_ihcT, W_hhT):
        w[..., 3 * HIDDEN:] *= 2.0
    b_eff[3 * HIDDEN:] *= 2.0

    # shared (weight) tensors, partition-folded + bf16
    shared = {
        "wfc": _bf16(_part_fold(W_fc.T.astype(f64), FC_IN)),      # [128,16,256]
        "w1": _bf16(_part_fold(W1, EMBED)),                       # [128,2,2048]
        "watteT": _bf16(_part_fold(np.ascontiguousarray(W_att_e.T), HIDDEN)),
        "wihcT": _bf16(_part_fold(W_ihcT, EMBED)),                # [128,2,2048]
        "whh": _bf16(_part_fold(W_hhT, HIDDEN)),                  # [128,4,2048]
        "wout": _bf16(_part_fold(np.ascontiguousarray(W_out.T.astype(f64)),
                                 HIDDEN)),                        # [128,4,1004]
        "beff": _bf16(b_eff[None, :]),                            # [1,2048]
        "ones1": _bf16(np.ones((1, BS))),                         # [1,32]
        "bfc": _f32(b_fc.reshape(2, 128).T.copy()),               # [128,2]
        "bout": _f32(np.concatenate([b_out, np.zeros(1024 - VOCAB,
                                                     b_out.dtype)]
                                    ).reshape(8, 128).T.copy()),  # [128,8]
    }
    i32 = np.zeros((128, BS), np.float32)
    for j in range(4):
        i32[j * 32:(j + 1) * 32] = np.eye(32)
    shared["i32"] = _bf16(i32)
    shared["ident32"] = _bf16(np.eye(32))

    in_maps = []
    for c in range(NCORES):
        sl = slice(c * BS, (c + 1) * BS)
        img_t = images[sl].T.astype(np.float64)                   # [2048, 32]
        caps = captions[sl]                                       # [32,T,512]
        # capsT[t] = caps[:, t, :].T -> [T,128,4,32] -> pair timesteps:
        # [T/2, 128, 2, 4, 32] (512B contiguous per partition per DMA)
        capsT = caps.transpose(1, 2, 0).reshape(T, 4, 128, BS)
        capsT = capsT.transpose(0, 2, 1, 3).reshape(T // 2, 2, 128, 4, BS)
        capsT = capsT.transpose(0, 2, 1, 3, 4)
        m = dict(shared)
        m["imagesT"] = _bf16(_part_fold(img_t, FC_IN))            # [128,16,32]
        m["capsT"] = _bf16(capsT)                                 # [T/2,128,2,4,32]
        in_maps.append(m)
    return in_maps


def build_nc():
    nc = bacc.Bacc("TRN2", target_bir_lowering=False)

    # --- dram params (inputs) ---
    d_imagesT = nc.declare_dram_parameter("imagesT", [128, 16, BS], BF16, isOutput=False)
    d_capsT = nc.declare_dram_parameter("capsT", [T // 2, 128, 2, 4, BS], BF16, isOutput=False)
    d_wfc = nc.declare_dram_parameter("wfc", [128, 16, EMBED], BF16, isOutput=False)
    d_w1 = nc.declare_dram_parameter("w1", [128, 2, G4], BF16, isOutput=False)
    d_watteT = nc.declare_dram_parameter("watteT", [128, 4, EMBED], BF16, isOutput=False)
    d_wihcT = nc.declare_dram_parameter("wihcT", [128, 2, G4], BF16, isOutput=False)
    d_whh = nc.declare_dram_parameter("whh", [128, 4, G4], BF16, isOutput=False)
    d_wout = nc.declare_dram_parameter("wout", [128, 4, VOCAB], BF16, isOutput=False)
    d_beff = nc.declare_dram_parameter("beff", [1, G4], BF16, isOutput=False)
    d_ones1 = nc.declare_dram_parameter("ones1", [1, BS], BF16, isOutput=False)
    d_bfc = nc.declare_dram_parameter("bfc", [128, 2], F32, isOutput=False)
    d_bout = nc.declare_dram_parameter("bout", [128, 8], F32, isOutput=False)
    d_i32 = nc.declare_dram_parameter("i32", [128, BS], BF16, isOutput=False)
    d_ident32 = nc.declare_dram_parameter("ident32", [32, 32], BF16, isOutput=False)

    d_out = nc.declare_dram_parameter("outT", [VOCAB, T * BS], F32, isOutput=True)

    with tile.TileContext(nc) as tc:
        with (
            tc.tile_pool(name="weights", bufs=1) as wpool,
            tc.tile_pool(name="consts", bufs=1) as cpool,
            tc.tile_pool(name="caps", bufs=8) as cappool,
            tc.tile_pool(name="cell", bufs=4) as cellpool,
            tc.tile_pool(name="psg", bufs=2, space="PSUM") as psg,
            tc.tile_pool(name="pst", bufs=2, space="PSUM") as pst,
            tc.tile_pool(name="psc", bufs=2, space="PSUM") as psc,
            tc.tile_pool(name="outsb", bufs=6) as opool,
        ):
            # --- load weights/constants into SBUF ---
            sb_wfc = wpool.tile([128, 16, EMBED], BF16)
            sb_w1 = wpool.tile([128, 2, G4], BF16)
            sb_watteT = wpool.tile([128, 4, EMBED], BF16)
            sb_wihcT = wpool.tile([128, 2, G4], BF16)
            sb_whh = wpool.tile([128, 4, G4], BF16)
            sb_wout = wpool.tile([128, 4, VOCAB], BF16)
            sb_imgT = wpool.tile([128, 16, BS], BF16)
            sb_beff = cpool.tile([1, G4], BF16)
            sb_ones1 = cpool.tile([1, BS], BF16)
            sb_bfc = cpool.tile([128, 2], F32)
            sb_bout = cpool.tile([128, 8], F32)
            sb_i32 = cpool.tile([128, BS], BF16)
            sb_id32 = cpool.tile([32, 32], BF16)
            # ordered so tensors needed first (phase A, step 0) land first;
            # whh is not needed until t=1, wout until t=15
            for dst, src in [(sb_imgT, d_imagesT), (sb_wfc, d_wfc),
                             (sb_bfc, d_bfc), (sb_beff, d_beff),
                             (sb_ones1, d_ones1), (sb_i32, d_i32),
                             (sb_id32, d_ident32), (sb_w1, d_w1),
                             (sb_watteT, d_watteT), (sb_wihcT, d_wihcT),
                             (sb_whh, d_whh), (sb_wout, d_wout),
                             (sb_bout, d_bout)]:
                nc.sync.dma_start(out=dst[:], in_=src[:])

            # --- phase A: featsT [256, 32] = W_fc @ images_s.T  (2 M-tiles)
            sb_featsT = cpool.tile([128, 2, BS], BF16)
            for m in range(2):
                ps = psg.tile([128, HIDDEN], F32, tag="ps0")
                for k in range(16):
                    nc.tensor.matmul(
                        ps[:, :BS],
                        lhsT=sb_wfc[:, k, m * 128:(m + 1) * 128],
                        rhs=sb_imgT[:, k, :],
                        start=(k == 0), stop=(k == 15),
                    )
                # += b_fc broadcast along free dim (instructions carrying a
                # scalar/bias pointer have a single sync-wait slot in the ISA,
                # so use plain tensor_tensor with a stride-0 broadcast AP)
                nc.vector.tensor_add(
                    sb_featsT[:, m, :], ps[:, :BS],
                    sb_bfc[:, m:m + 1].broadcast_to((128, BS)),
                )

            # --- phase A2: gx0 [128(4gx32b), 512] = b_eff + feats @ W1
            sb_gx0 = cpool.tile([128, HIDDEN], BF16)
            ps0 = psg.tile([128, HIDDEN], F32, tag="ps0")
            for j in range(4):
                osl = ps0[32 * j:32 * (j + 1), :]
                nc.tensor.matmul(
                    osl, lhsT=sb_ones1[:, :],
                    rhs=sb_beff[:, j * HIDDEN:(j + 1) * HIDDEN],
                    start=True, stop=False, tile_position=(0, 32 * j),
                )
                for k in range(2):
                    nc.tensor.matmul(
                        osl, lhsT=sb_featsT[:, k, :],
                        rhs=sb_w1[:, k, j * HIDDEN:(j + 1) * HIDDEN],
                        start=False, stop=(k == 1), tile_position=(0, 32 * j),
                    )
            nc.vector.tensor_copy(sb_gx0[:], ps0[:])

            # --- phase B: the 128-step recurrence ---
            # hs_all holds hT for every step: [128, T, 4, 32] (hidden chunks
            # on partitions, (t, chunk, batch) on free)
            sb_hs = wpool.tile([128, T, 4, BS], BF16)
            sb_c = cpool.tile([64, HIDDEN], BF16)
            nc.vector.memset(sb_c[32:64, :], 0.0)

            out_units = [(n, m) for n in range(NT_CHUNKS)
                         for m in range(VOCAB_TILES)]

            def emit_out_unit(nchunk, m):
                tspan = slice(nchunk * (T // NT_CHUNKS),
                              (nchunk + 1) * (T // NT_CHUNKS))
                mv = 128 if m < 7 else VOCAB - 7 * 128
                msl = slice(m * 128, m * 128 + mv)
                ps_o = psc.tile([128, 512], F32, tag="ctx")
                for k in range(4):
                    nc.tensor.matmul(
                        ps_o[:mv, :],
                        lhsT=sb_wout[:, k, msl],
                        rhs=sb_hs[:, tspan, k, :],
                        start=(k == 0), stop=(k == 3),
                    )
                sb_o = opool.tile([128, 512], F32)
                # ScalarE Identity+bias: b_out is per-partition here (vocab
                # on partitions); keeps the eviction off DVE, which carries
                # the chain's cell ops (Bacc legalizes the extra sync waits)
                nc.scalar.activation(
                    sb_o[:mv, :], ps_o[:mv, :],
                    mybir.ActivationFunctionType.Identity,
                    bias=sb_bout[:mv, m:m + 1],
                )
                dma_eng = nc.sync if (m + nchunk) % 2 == 0 else nc.scalar
                dma_eng.dma_start(
                    out=d_out[msl, nchunk * 512:(nchunk + 1) * 512],
                    in_=sb_o[:mv, :])

            hT_prev = None
            for t in range(T):
                if t % 2 == 0:
                    sb_cap2 = cappool.tile([128, 2, 4, BS], BF16)
                    nc.sync.dma_start(out=sb_cap2[:], in_=d_capsT[t // 2])
                if t % 2 == 0:
                    # stage 1 for the timestep pair: ctx_eT [256, 2, 32] =
                    # W_att_e @ emb_{t,t+1}.T (rank-256 factorization of the
                    # folded caption->gates product)
                    ps_cx = psc.tile([128, 2, 2, BS], F32, tag="ctx")
                    for m in range(2):
                        for k in range(4):
                            nc.tensor.matmul(
                                ps_cx[:, m, :, :],
                                lhsT=sb_watteT[:, k, m * 128:(m + 1) * 128],
                                rhs=sb_cap2[:, :, k, :],
                                start=(k == 0), stop=(k == 3),
                            )
                    sb_cx2 = cellpool.tile([128, 2, 2, BS], BF16, tag="cx")
                    nc.scalar.copy(sb_cx2[:], ps_cx[:])
                sb_cx = sb_cx2[:, :, t % 2]

                # Split the gates by hidden-half so sigma/cell/transpose of
                # half A overlap the PE rounds and cell of half B, and the
                # next step's h-rounds k=0,1 can start as soon as half A's
                # hT chunks are stored.
                HH = HIDDEN // 2  # 256
                ps_half = [None, None]
                for x in range(2):
                    hsl = slice(x * HH, (x + 1) * HH)  # hidden half
                    ps = psg.tile([128, HH], F32, tag=f"ps{x}")
                    ps_half[x] = ps
                    for j in range(4):
                        osl = ps[32 * j:32 * (j + 1), :]
                        gsl = slice(j * HIDDEN + x * HH,
                                    j * HIDDEN + (x + 1) * HH)
                        # seed: gates_x feats part + bias (identity matmul)
                        nc.tensor.matmul(
                            osl, lhsT=sb_i32[32 * j:32 * (j + 1), :],
                            rhs=sb_gx0[32 * j:32 * (j + 1), hsl],
                            start=True, stop=False,
                            tile_position=(32 * j, 32 * j),
                        )
                        # stage 2: ctx_e @ W_ih_c.T rounds (h-independent)
                        for k in range(2):
                            nc.tensor.matmul(
                                osl, lhsT=sb_cx[:, k, :],
                                rhs=sb_wihcT[:, k, gsl],
                                start=False, stop=(t == 0 and k == 1),
                                tile_position=(0, 32 * j),
                            )
                        # recurrent rounds
                        if t > 0:
                            for k in range(4):
                                nc.tensor.matmul(
                                    osl, lhsT=hT_prev[:, k, :],
                                    rhs=sb_whh[:, k, gsl],
                                    start=False, stop=(k == 3),
                                    tile_position=(0, 32 * j),
                                )

                for x in range(2):
                    hsl = slice(x * HH, (x + 1) * HH)
                    ps = ps_half[x]
                    # cell (per hidden-half). DVE tensor_tensor requires both
                    # SBUF inputs at the same base partition, so align
                    # operands by quadrant: sigma(i,f,o) at quadrants 0,1,2;
                    # c at q1; tanh(g) lands at q0; tanh(c) at q2.
                    acts = cellpool.tile([128, HH], BF16, tag=f"acts{x}")
                    nc.scalar.activation(acts[:, :], ps[:, :],
                                         mybir.ActivationFunctionType.Sigmoid)
                    tgt = cellpool.tile([32, HH], BF16, tag=f"tgt{x}")
                    tct = cellpool.tile([96, HH], BF16, tag=f"tct{x}")
                    # tanh(g) = 2*sigmoid(2g) - 1 (g rows pre-scaled by 2);
                    # single-src op, cross-quadrant write q3 -> q0
                    nc.vector.tensor_scalar(
                        out=tgt[0:32, :], in0=acts[96:128, :],
                        scalar1=2.0, scalar2=-1.0,
                        op0=mybir.AluOpType.mult, op1=mybir.AluOpType.add,
                    )
                    t1 = cellpool.tile([64, HH], BF16, tag=f"t1{x}")
                    nc.vector.tensor_mul(t1[32:64, :], acts[0:32, :],
                                         tgt[0:32, :])
                    t2 = cellpool.tile([64, HH], BF16, tag=f"t2{x}")
                    nc.vector.tensor_mul(t2[32:64, :], acts[32:64, :],
                                         sb_c[32:64, hsl])
                    nc.vector.tensor_add(sb_c[32:64, hsl], t1[32:64, :],
                                         t2[32:64, :])
                    nc.scalar.activation(tct[64:96, :], sb_c[32:64, hsl],
                                         mybir.ActivationFunctionType.Tanh)
                    h = cellpool.tile([32, HH], BF16, tag=f"h{x}")
                    nc.vector.tensor_mul(h[:], acts[64:96, :], tct[64:96, :])

                    # transpose h half -> hT chunks 2x, 2x+1 of hs_all[t]
                    ps_t = pst.tile([128, 2, BS], BF16, tag="pst")
                    for k in range(2):
                        nc.tensor.transpose(ps_t[:, k, :],
                                            h[:, 128 * k:128 * (k + 1)],
                                            sb_id32[:, :])
                    nc.vector.tensor_copy(sb_hs[:, t, 2 * x:2 * x + 2, :], ps_t[:])
                hT_prev = sb_hs[:, t]

                # phase C interleaved: the output chunk for timesteps
                # [16n, 16n+16) only needs hs rows written so far. Emit one
                # (chunk, m-tile) unit per step so its matmuls fill PE idle
                # slots in the chain without hogging psum slots.
                while out_units and out_units[0][0] * (T // NT_CHUNKS) + 15 <= t:
                    nchunk, m = out_units.pop(0)
                    emit_out_unit(nchunk, m)
                    if t < T - 1:
                        break
    nc.compile()
    return nc


_NC_CACHE = None


def kernel(**inputs) -> np.ndarray:
    global _NC_CACHE
    from concourse.bass_utils import run_bass_kernel_spmd

    in_maps = prepare_host(**inputs)
    if _NC_CACHE is None:
        _NC_CACHE = build_nc()
    nc = _NC_CACHE
    res = run_bass_kernel_spmd(nc, in_maps, list(range(NCORES)))
    outs = []
    for c in range(NCORES):
        o = res.results[c]["outT"]  # [1004, 4096] f32, free = (t, b)
        o = o.reshape(VOCAB, T, BS).transpose(2, 1, 0)  # [32, T, 1004]
        outs.append(o)
    return np.ascontiguousarray(np.concatenate(outs, axis=0).astype(np.float32))


if __name__ == "__main__":
    nc = build_nc()
    print("built ok")
